# revision 43
# baseline (speedup 1.0000x reference)
"""Trainium2 Bass kernel for nn_DCM (dynamic conv module), data-parallel over
batch N=8 across 8 NeuronCores (1 sample per core).

Per-core program (sample n):
  x [512, 3600] bf16 (host-cast) in chunk-major layout
  for k in (1,3,5):
    f_k = relu(w1k' @ x + b1k)          (1x1 conv, BN scale folded into w)
    pooled_k = block-sums of x          (chunkwise 4x4-block DVE reductions,
                                         1/area folded into w2)
    g_k = relu(w2k'' @ pooled_k + b2k)  (tiny matmul)
    o_k = relu(depthwise(f_k, g_k))     (k^2 diag(g) matmuls on shifted
                                         zero-padded windows, PSUM accum;
                                         k=1 is a fused scale+relu on ACT)
    d_k = relu(wfk' @ o_k + bfk)
  y = relu(w_out' @ [x;d1;d3;d5] + b_out)  (16 K-tiles accumulated in PSUM)

Precision: x / f / o / w1 / w2 / wf and the out-conv x-part run bf16;
the depthwise diag matmuls (f3/f5 + diag(g) in fp8e4) and the out-conv
d-part K-tiles (d tiles + their w_out columns in fp8e4) run fp8 with
DoubleRow perf mode, pairing two 128-deep K-tiles per instruction at
2x PE throughput (measured 183ns vs 2x191ns per pair). fp32 PSUM
accumulate everywhere; measured end-to-end rel_l2 ~9e-3 vs the fp32
reference (gate 2e-2). y is stored bf16 and upcast on the host.
"""

import json

import numpy as np
import ml_dtypes

import concourse.bass as bass
import concourse.tile as tile
from concourse import mybir
from concourse.vector_clock import ScopedClock

P = 128
C = 512
C4 = 128
H = W = 60
HW = H * W
NB = 10          # bands
BR = 6           # rows per band
NT = BR * W      # 360 columns per band
CHUNK = 2 * NT   # x DMA chunk = 2 bands
NCHUNK = HW // CHUNK
CROWS = CHUNK // W  # rows per chunk (12)
N_CORES = 8
F32 = mybir.dt.float32
BF16 = mybir.dt.bfloat16
FP8 = mybir.dt.float8e4
DR = mybir.MatmulPerfMode.DoubleRow
RELU = mybir.ActivationFunctionType.Relu

# ---------------------------------------------------------------------------
# Patches for walrus/concourse skew in this container: this walrus build only
# encodes ONE sync wait per instruction, while Tile emits several.
# 1) TileContext tail drain: emit its waits as 1-wait NOPs on SP instead.
# 2) to_json_bytes post-pass: split any instruction with N>1 waits into N-1
#    preceding same-engine 1-wait NOPs (same-engine program order makes this
#    semantically identical).
# ---------------------------------------------------------------------------


def _patched_drain_and_barrier(self, tick_clock, wait_clock):
    nc = self.nc
    probe = nc.sync.nop(nofuse=True)
    wait_clock.add_sem_waits(probe.ins, ScopedClock({None: tick_clock.global_clock}))
    si = probe.ins.sync_info
    waits = list(si.on_wait) if si is not None else []
    probe.ins.sync_info = mybir.SyncInfo(on_wait=[], on_update=list(si.on_update))

    # distribute the global-clock waits engine-affine (1-wait NOPs), then the
    # all-engine barrier transitively covers everything
    def eng_for(w):
        name = getattr(w, "ant_name", None) or ""
        if name.startswith("Activation"):
            return nc.scalar
        if name.startswith("DVE"):
            return nc.vector
        if name.startswith("PE"):
            return nc.tensor
        if name.startswith("Pool") or name.startswith("DMASW"):
            return nc.gpsimd
        return nc.sync

    for w in waits:
        n = eng_for(w).nop(nofuse=True)
        n.ins.sync_info = mybir.SyncInfo(on_wait=[w], on_update=[])
    nc.sync.drain()
    nc.all_engine_barrier()
    assert self.sems is not None
    popped = nc._tile_sem_poison_stack.pop()
    assert popped is self._sem_poison
    # Skip emitting the tail sem-clear/dma-reset instructions + second barrier
    # (~7us): the program preamble re-initializes semaphores on each
    # execution. Keep the allocator bookkeeping that clear_and_free did.
    sems = list(self.sems.allocated().values())
    sem_nums = [s.num if hasattr(s, "num") else s for s in sems]
    if sem_nums:
        nc._state.prepend_free_semaphores(sem_nums)
        for poison_set in nc._tile_sem_poison_stack:
            poison_set.update(sem_nums)


def _split_waits_json(raw: bytes) -> bytes:
    m = json.loads(raw)
    ctr = 0
    changed = False
    for f in m.get("functions", []):
        for bb in f.get("blocks", []):
            if bb.get("name", "").endswith("_end"):
                # end-of-program drains don't need walrus's per-sem reset
                # expansion (~5us serial); the start-of-program drains
                # already reset the kernel sem range on every execution
                for inst in bb.get("instructions", []):
                    if inst.get("opcode") == "Drain":
                        inst["is_reset_sema"] = False
                        changed = True
            out = []
            for inst in bb.get("instructions", []):
                si = inst.get("sync_info")
                waits = (si or {}).get("on_wait") or []
                if len(waits) > 1:
                    changed = True
                    for w in waits[:-1]:
                        ctr += 1
                        nop = {
                            "engine": inst.get("engine"),
                            "ins": [],
                            "outs": [],
                            "name": f"{inst['name']}-sw{ctr}",
                            "opcode": "NoOp",
                            "sync_info": {"on_update": [], "on_wait": [w]},
                        }
                        if "debug" in inst:
                            nop["debug"] = inst["debug"]
                        out.append(nop)
                    si["on_wait"] = [waits[-1]]
                out.append(inst)
            bb["instructions"] = out
    return json.dumps(m).encode() if changed else raw


_PATCHED = False


def _apply_patches():
    global _PATCHED
    if _PATCHED:
        return
    tile.TileContext._drain_and_barrier = _patched_drain_and_barrier
    orig = bass.Bass.to_json_bytes

    def _patched_to_json_bytes(self, *a, **kw):
        return _split_waits_json(orig(self, *a, **kw))

    bass.Bass.to_json_bytes = _patched_to_json_bytes
    _PATCHED = True


# ---------------------------------------------------------------------------
# Bass program
# ---------------------------------------------------------------------------


def _build_bass():
    _apply_patches()
    nc = bass.Bass(trn_type="TRN2")

    # all inputs pre-arranged on host into partition-major layouts
    # partition-major so per-partition DMA lines are up to 14400B contiguous
    x_d = nc.dram_tensor("x", [P, NCHUNK, 4, CHUNK], BF16, kind="ExternalInput")
    x8_d = nc.dram_tensor("x8", [P, NCHUNK, 4, CHUNK], FP8, kind="ExternalInput")
    # block-transposed pooling copy: [P, kt, e(4x4-block elem), blk(hb*15+wb)]
    # moved as two halves (7200B/partition lines -> ~2x queue rate vs 3600B)
    xp_d = nc.dram_tensor("xp", [P, 4, 16, 225], FP8, kind="ExternalInput")
    w1_d = nc.dram_tensor("w1sb", [P, 3, 2, 2, C4], FP8, kind="ExternalInput")
    w2_d = nc.dram_tensor("w2sb", [P, 4, 3, C4], BF16, kind="ExternalInput")
    wf_d = nc.dram_tensor("wfsb", [P, 3, C], BF16, kind="ExternalInput")
    wo_d = nc.dram_tensor("wosb", [P, 4, C], BF16, kind="ExternalInput")
    wod8_d = nc.dram_tensor("wod8", [P, 3, 2, 2, C], FP8, kind="ExternalInput")
    b1_d = nc.dram_tensor("b1sb", [P, 3], F32, kind="ExternalInput")
    b2_d = nc.dram_tensor("b2sb", [P, 3], F32, kind="ExternalInput")
    bf_d = nc.dram_tensor("bfsb", [P, 3, 4], F32, kind="ExternalInput")
    bo_d = nc.dram_tensor("bosb", [P, 4], F32, kind="ExternalInput")
    id_d = nc.dram_tensor("ident", [P, P], BF16, kind="ExternalInput")
    # partition-major y so grouped stores have multi-KB per-partition lines;
    # band 9 gets its own [P, 4*NT]-contiguous tensor for a fast tail drain
    y_d = nc.dram_tensor("y", [P, 4, NB - 1, NT], BF16, kind="ExternalOutput")
    y9_d = nc.dram_tensor("y9", [P, 4, NT], BF16, kind="ExternalOutput")

    with tile.TileContext(nc) as tc:
        with (
            tc.tile_pool(name="consts", bufs=1) as consts,
            tc.tile_pool(name="xpool", bufs=1) as xpool,
            tc.tile_pool(name="fpool", bufs=1) as fpool,
            tc.tile_pool(name="ptmp", bufs=2) as ptmp,
            tc.tile_pool(name="gpool", bufs=1) as gpool,
            tc.tile_pool(name="obuf", bufs=3) as obuf,
            tc.tile_pool(name="dbuf", bufs=3) as dbuf,
            tc.tile_pool(name="ybuf", bufs=3) as ybuf,
            tc.tile_pool(name="psum", bufs=4, space="PSUM") as psum,
        ):
            # ---- PE HAM warm-up: dummy matmuls on zeroed SBUF so the clock
            # gate opens + pstate ramps before the first real matmul; the fp8
            # x chunk lands before the PE preamble finishes, so just enough
            # to cover the ramp ----
            warm = consts.tile([P, C], BF16, name="warmup")
            nc.vector.memset(warm[:], 0.0)
            wps = psum.tile([P, C], F32, tag="work", name="warmps")
            for _ in range(12):
                nc.tensor.matmul(wps[:], warm[:, 0:P], warm[:],
                                 start=True, stop=True)

            # ---- weights / constants -> SBUF ----
            # The two HWDGE queues (sync + scalar) carry the bulk tensors in
            # need-order; the gpsimd SW queue (~76 B/ns, otherwise idle)
            # carries all weights/biases that are small or needed mid-stream.
            # gpsimd's SW DGE consumes the Pool engine itself, and Pool now
            # runs half the pooling tree — keep Q0 to the tiny biases only
            b1 = consts.tile([P, 3], F32)
            nc.gpsimd.dma_start(b1[:], b1_d[:])
            b2 = consts.tile([P, 3], F32)
            nc.gpsimd.dma_start(b2[:], b2_d[:])
            ident = consts.tile([P, P], BF16)
            nc.gpsimd.dma_start(ident[:], id_d[:])
            bfb = consts.tile([P, 3, 4], F32)
            nc.gpsimd.dma_start(bfb[:], bf_d[:])
            bo = consts.tile([P, 4], F32)
            nc.gpsimd.dma_start(bo[:], bo_d[:])

            # ---- x -> SBUF (chunk-major, contiguous per partition) ----
            # Queue order = arrival order. Critical chain: x8 chunks (f convs)
            # + xp halves (pooling tree -> g -> diag) first, then bf16 x chunk
            # 0 + out-conv weights (band-0 start), then trailing bf16 x.
            x8_sb = xpool.tile([P, NCHUNK, 4, CHUNK], FP8)
            x_sb = xpool.tile([P, NCHUNK, 4, CHUNK], BF16)
            xp_sb = xpool.tile([P, 4, 16, 225], FP8, name="xp")
            woT = consts.tile([P, 4, C], BF16)
            wfT = consts.tile([P, 3, C], BF16)
            wod8 = consts.tile([P, 3, 2, 2, C], FP8)
            w1T = consts.tile([P, 3, 2, 2, C4], FP8)
            w2T = consts.tile([P, 4, 3, C4], BF16)
            nc.scalar.dma_start(w1T[:], w1_d[:])
            nc.sync.dma_start(xp_sb[:, 0:2], xp_d[:, 0:2])
            nc.scalar.dma_start(xp_sb[:, 2:4], xp_d[:, 2:4])
            nc.sync.dma_start(x8_sb[:, 0:1], x8_d[:, 0:1])
            nc.sync.dma_start(x8_sb[:, 1:3], x8_d[:, 1:3])
            nc.scalar.dma_start(w2T[:], w2_d[:])
            nc.sync.dma_start(woT[:, 0:2], wo_d[:, 0:2])
            nc.scalar.dma_start(wfT[:], wf_d[:])
            nc.sync.dma_start(x_sb[:, 0:1], x_d[:, 0:1])
            nc.scalar.dma_start(wod8[:], wod8_d[:])
            nc.sync.dma_start(x_sb[:, 1:3], x_d[:, 1:3])
            nc.scalar.dma_start(woT[:, 2:4], wo_d[:, 2:4])
            nc.scalar.dma_start(x8_sb[:, 3:5], x8_d[:, 3:5])
            nc.scalar.dma_start(x_sb[:, 3:5], x_d[:, 3:5])

            def xsl(kt, b):
                """bf16 x band slice [P, NT] for band b, K-tile kt."""
                return x_sb[:, b // 2, kt, (b % 2) * NT:(b % 2) * NT + NT]

            def x8sl(q, b):
                """fp8 x band pair [P, 2, NT] for band b, K-tiles 2q:2q+2."""
                c0 = (b % 2) * NT
                return x8_sb[:, b // 2, 2 * q:2 * q + 2, c0:c0 + NT]

            # ---- f convs (k=1 plain, k=3/5 zero-padded fp8 layouts) ----
            # band-outer so each arriving x chunk feeds 3 convs' worth of PE
            f1 = fpool.tile([P, HW], BF16)
            f3 = fpool.tile([P, 64, 64], FP8)
            f5 = fpool.tile([P, 64, 64], FP8)
            for fpad in (f3, f5):  # zero only the halo border strips (on the
                # Pool engine so the DVE pooling tree starts immediately)
                nc.gpsimd.memset(fpad[:, 0:2, :], 0.0)
                nc.gpsimd.memset(fpad[:, 62:64, :], 0.0)
                nc.gpsimd.memset(fpad[:, 2:62, 0:2], 0.0)
                nc.gpsimd.memset(fpad[:, 2:62, 62:64], 0.0)
            F_CONVS = ((0, f1), (1, f3), (2, f5))

            def emit_f_conv(b, ci):
                ki, fdst = F_CONVS[ci]
                ps = psum.tile([P, NT], F32, tag="work", name=f"fps{b}{ki}")
                for q in range(2):
                    nc.tensor.matmul(ps[:], w1T[:, ki, q, :, :], x8sl(q, b),
                                     start=(q == 0), stop=(q == 1),
                                     perf_mode=DR, skip_group_check=True)
                if ki == 0:
                    dst = fdst[:, b * NT:(b + 1) * NT]
                else:
                    dst = fdst[:, 2 + b * BR: 2 + (b + 1) * BR, 2:62]
                with nc.allow_low_precision(reason="fp8 f tiles"):
                    nc.scalar.activation(dst, ps[:], RELU,
                                         bias=b1[:, ki:ki + 1], scale=1.0)

            def emit_f_band(b):
                for ci in range(3):
                    emit_f_conv(b, ci)

            # only the first f bands run before the band loop (taps for band b
            # need f bands b-1..b+1); the rest are emitted inside the band
            # loop so band 0 starts as soon as the pooling->g->diag chain is
            # ready instead of after the whole f block
            for b in range(3):
                emit_f_band(b)

            # ---- pooling stage 1: flat dense halves-add fold tree (DVE's
            # 2x/4x fast modes need flat dense operands; tensor_reduce is
            # stuck at 1x) over the block-transposed copy:
            # [16, 225] -> 4x4 block sums q4 [P, 4, 225] ----
            pooled = {k: gpool.tile([P, 4, k * k], BF16, name=f"pooled{k}")
                      for k in (1, 3, 5)}
            q4 = gpool.tile([P, 4, 225], F32, name="q4")

            def flat(apv, n):
                """Collapse a contiguous free region to one [1, n] AP dim —
                the DVE 2x/4x fast modes demote on multi-dim APs."""
                w = apv.copy()
                a = w.ap
                while len(a) > 1:
                    a.pop()
                a.append((1, n))
                w.ap = a
                return w

            def tree_chain(kt, eng, tag):
                a = ptmp.tile([P, 1800], BF16, tag=f"s1{tag}")
                b_ = ptmp.tile([P, 900], BF16, tag=f"s2{tag}")
                c_ = ptmp.tile([P, 450], BF16, tag=f"s3{tag}")
                eng.tensor_tensor(
                    a[:], flat(xp_sb[:, kt, 0:8, :], 1800),
                    flat(xp_sb[:, kt, 8:16, :], 1800),
                    mybir.AluOpType.add)
                eng.tensor_tensor(
                    b_[:], a[:, 0:900], a[:, 900:1800],
                    mybir.AluOpType.add)
                eng.tensor_tensor(
                    c_[:], b_[:, 0:450], b_[:, 450:900],
                    mybir.AluOpType.add)
                eng.tensor_tensor(
                    q4[:, kt, :], c_[:, 0:225], c_[:, 225:450],
                    mybir.AluOpType.add)

            def emit_pool_kt(k, kt, eng):
                # one fused XY reduce: q4 (hb,wb) -> pooled[k] [i][j]
                with nc.allow_low_precision(reason="pooled block sums in bf16"):
                    eng.reduce_sum(
                        pooled[k][:, kt, :].rearrange(
                            "p (i j) -> p i j", i=k),
                        q4[:, kt].rearrange(
                            "p (hbB hb wbB wb) -> p hbB wbB hb wb",
                            hbB=k, hb=15 // k, wbB=k),
                        axis=mybir.AxisListType.XY)

            # chains split DVE (kt0, kt2) / Pool (kt1, kt3) so the tree's
            # ~3.4us/chain serial latency halves; reduces (DVE-only op)
            # interleave so pooled3 (the earliest PE consumer) completes
            # first; pooled1 re-sums pooled3's 9 values instead of q4's 225
            with nc.allow_low_precision(reason="bf16 pooling tree"):
                tree_chain(0, nc.vector, "v")
                tree_chain(1, nc.gpsimd, "p")
                emit_pool_kt(3, 0, nc.vector)
                emit_pool_kt(5, 0, nc.vector)
                tree_chain(2, nc.vector, "v")
                tree_chain(3, nc.gpsimd, "p")
                emit_pool_kt(3, 1, nc.vector)
                emit_pool_kt(5, 1, nc.vector)
                emit_pool_kt(3, 2, nc.vector)
                emit_pool_kt(5, 2, nc.vector)
                emit_pool_kt(3, 3, nc.vector)
                for kt in range(4):
                    nc.vector.reduce_sum(
                        pooled[1][:, kt, :], pooled[3][:, kt, :],
                        axis=mybir.AxisListType.X)
                emit_pool_kt(5, 3, nc.vector)

            g_sb = {}
            diag = {}

            def emit_g(k, ki, diag_engine=None):
                gp = psum.tile([P, k * k], F32, tag="work", name=f"gp{k}")
                for kt in range(4):
                    nc.tensor.matmul(gp[:], w2T[:, kt, ki, :], pooled[k][:, kt, :],
                                     start=(kt == 0), stop=(kt == 3))
                g = gpool.tile([P, k * k], F32, name=f"g{k}")
                # 1/pool-area folded into the activation scale (keeps w2
                # magnitudes DMA-layout-splittable without fp8 underflow)
                area = (H // k) * (W // k)
                nc.scalar.activation(g[:], gp[:], RELU,
                                     bias=b2[:, ki:ki + 1], scale=1.0 / area)
                g_sb[k] = g
                if diag_engine is not None:
                    # diag tiles via broadcast multiply into ONE [P, k*k, P]
                    # fp8 tile (pairs of taps slice naturally for DoubleRow),
                    # built in <=13-tap pieces so the first taps can start
                    # before the whole set: dg[p, t, c] = ident[p,c] * g[p,t]
                    dg = gpool.tile([P, k * k, P], FP8, name=f"diag{k}")
                    t0 = 0
                    while t0 < k * k:
                        n = min(13, k * k - t0)
                        with nc.allow_low_precision(reason="fp8 diag taps"):
                            diag_engine.tensor_tensor(
                                dg[:, t0:t0 + n, :],
                                ident[:, None, :].to_broadcast((P, n, P)),
                                g[:, t0:t0 + n, None].to_broadcast((P, n, P)),
                                mybir.AluOpType.mult)
                        t0 += n
                    diag[k] = dg

            # only g3 pre-loop (earliest consumer: band-0 k=3 taps); g5 and
            # g1 are emitted inside band 0 between its tap sections so the
            # PE isn't blocked on the slower k=5/k=1 pooling chains
            emit_f_band(3)
            emit_g(3, 1, nc.vector)

            # ---- band loop ----
            ystage = [None]
            for b in range(NB):
                # emit a future f band (band b+1's taps need f band b+2 done;
                # emitting f(b+4) in band b keeps 2 bands of slack); band 0
                # instead weaves f4 into its wf section, since at T0 the x
                # chunks/weights for the woven x-MMs haven't landed yet
                if 0 < b and b + 4 < NB:
                    emit_f_band(b + 4)
                # depthwise taps (k=3, k=5) accumulate in PSUM
                o_sb = {}
                for k, fpad in ((3, f3), (5, f5)):
                    if b == 0 and k == 5:
                        # g5's chain (Pool-side reduces) lands just before
                        # band 0's k=5 taps need diag5
                        emit_g(5, 2, nc.vector)
                    pad = (k - 1) // 2
                    kk = k * k

                    def win(t):
                        r0 = 2 + b * BR + t // k - pad
                        c0 = 2 + t % k - pad
                        return r0 * 64 + c0, fpad[:, r0:r0 + BR, c0:c0 + W]

                    ps = psum.tile([P, NT], F32, tag="work")
                    # fp8 DoubleRow: two taps per matmul; rhs is the first
                    # tap's window with an inserted (delta, 2) AP dim
                    for q in range(kk // 2):
                        off0, w0 = win(2 * q)
                        off1, _ = win(2 * q + 1)
                        v = w0.ap
                        v.insert(1, (off1 - off0, 2))
                        w0.ap = v
                        nc.tensor.matmul(
                            ps[:], diag[k][:, 2 * q:2 * q + 2, :], w0,
                            start=(q == 0), stop=False,
                            perf_mode=DR, skip_group_check=True)
                    _, wlast = win(kk - 1)
                    nc.tensor.matmul(ps[:], diag[k][:, kk - 1, :], wlast,
                                     start=False, stop=True,
                                     skip_group_check=True)
                    o = obuf.tile([P, NT], BF16, tag=f"o{k}")
                    nc.scalar.activation(o[:], ps[:], RELU, bias=0.0, scale=1.0)
                    o_sb[k] = o
                if b == 0:
                    emit_g(1, 0)
                # k=1: o1 = relu(g1 * f1)
                o1 = obuf.tile([P, NT], BF16, tag="o1")
                nc.scalar.activation(o1[:], f1[:, b * NT:(b + 1) * NT], RELU,
                                     bias=0.0, scale=g_sb[1][:, 0:1])
                o_sb[1] = o1

                # final-conv accumulators; d-path first (wf + out-conv d-MM
                # beats), x-part MMs LAST so the bf16 x chunks are only
                # needed ~10us into each band (startup DMA off the critical
                # path); op accumulation: first d-MM starts, last x-MM stops
                op = [psum.tile([P, NT], F32, tag="out", name=f"op{b}_{m}")
                      for m in range(4)]
                d_tiles = {}

                def emit_wf_mm(pi, m):
                    k = (3, 5, 1)[pi]
                    if m == 0:
                        d_tiles[k] = dbuf.tile([P, 4, NT], FP8, tag="d",
                                               name=f"d{b}_{k}")
                    d_sb = d_tiles[k]
                    dps = psum.tile([P, NT], F32, tag="work",
                                    name=f"dps{b}{pi}{m}")
                    ki = {1: 0, 3: 1, 5: 2}[k]
                    nc.tensor.matmul(dps[:], wfT[:, ki, m * P:(m + 1) * P],
                                     o_sb[k][:], start=True, stop=True)
                    with nc.allow_low_precision(reason="fp8 d tiles"):
                        if m % 2 == 0:
                            nc.vector.tensor_scalar(
                                d_sb[:, m, :], dps[:],
                                bfb[:, ki, m:m + 1], 0.0,
                                op0=mybir.AluOpType.add, op1=mybir.AluOpType.max)
                        else:
                            nc.scalar.activation(
                                d_sb[:, m, :], dps[:], RELU,
                                bias=bfb[:, ki, m:m + 1], scale=1.0)

                # weave: the 12 wf-MMs interleaved with the m0/m1 x-part MMs
                # (independent work that hides the wf->epilogue->d latency),
                # then the 24 out-conv d-MMs with max slack from their wf
                # epilogues, then the m2/m3 x-part MMs; op[0]/op[1] finish at
                # the d block's end so their y epilogues overlap the x tail
                xw = 0 if b == 0 else 2  # m-tiles of x woven into wf section

                def emit_x_mm(m, kt):
                    nc.tensor.matmul(op[m], woT[:, kt, m * P:(m + 1) * P],
                                     xsl(kt, b), start=(m < xw and kt == 0),
                                     stop=(m >= xw and kt == 3),
                                     skip_group_check=True)

                xq = [(m, kt) for m in range(xw) for kt in range(4)]
                for j in range(12):
                    emit_wf_mm(*divmod(j, 4))
                    if b == 0 and j % 4 == 1:
                        emit_f_conv(4, j // 4)
                    if xq and j % 3 == 2:
                        emit_x_mm(*xq.pop(0))
                        emit_x_mm(*xq.pop(0))
                for pi, k in enumerate((3, 5, 1)):
                    wpi = {1: 0, 3: 1, 5: 2}[k]
                    d_sb = d_tiles[k]
                    for q in range(2):
                        for m in range(4):
                            nc.tensor.matmul(
                                op[m], wod8[:, wpi, q, :, m * P:(m + 1) * P],
                                d_sb[:, 2 * q:2 * q + 2, :],
                                start=(m >= xw and pi == 0 and q == 0),
                                stop=(m < xw and pi == 2 and q == 1),
                                perf_mode=DR, skip_group_check=True)
                for m in range(xw, 4):
                    for kt in range(4):
                        emit_x_mm(m, kt)
                # epilogue (alternate engines); y staged in SBUF in groups
                # (bands 0-4, 5-8, 9) and stored with multi-KB per-partition
                # lines -- per-band [P,360] stores only get 720B packets and
                # the queue's ~25ns/packet overhead made them the tail gate
                if b in (0, 5, 9):
                    ysb = ybuf.tile([P, 4, 5 if b == 0 else 4 if b == 5 else 1,
                                     NT], BF16, tag="y")
                    ystage[0] = (ysb, b)
                ysb, g0 = ystage[0]
                yc = b - g0
                for m in range(4):
                    if m % 2 == 0:
                        nc.vector.tensor_scalar(
                            ysb[:, m, yc, :], op[m], bo[:, m:m + 1], 0.0,
                            op0=mybir.AluOpType.add, op1=mybir.AluOpType.max)
                    else:
                        nc.scalar.activation(ysb[:, m, yc, :], op[m], RELU,
                                             bias=bo[:, m:m + 1], scale=1.0)
                if b in (4, 8):
                    for m in range(4):
                        dma_eng = nc.sync if m % 2 == 0 else nc.scalar
                        dma_eng.dma_start(y_d[:, m, g0:b + 1, :],
                                          ysb[:, m, :, :])
                elif b == 9:
                    nc.sync.dma_start(y9_d[:, 0:2, :], ysb[:, 0:2, 0, :])
                    nc.scalar.dma_start(y9_d[:, 2:4, :], ysb[:, 2:4, 0, :])
    return nc


# ---------------------------------------------------------------------------
# Host side
# ---------------------------------------------------------------------------

_NC_CACHE = {}


def _get_nc():
    if "nc" not in _NC_CACHE:
        _NC_CACHE["nc"] = _build_bass()
    return _NC_CACHE["nc"]


def _host_prep(inputs):
    """Fold BN scales into weights, transpose into partition-major SBUF
    layouts, cast bf16."""
    bf16 = ml_dtypes.bfloat16
    f32 = np.float32

    def A(name):
        return np.asarray(inputs[name], f32)

    # lhsT for conv1: [K=C, M=C4] per k, fp8 DoubleRow pairs of K-tiles
    # -> [P, 3, 2(pair), 2(sub), C4]
    w1T = np.stack([(A(f"s1_{k}")[:, None] * A(f"w1_{k}")).T for k in (1, 3, 5)])
    w1sb = w1T.reshape(3, 2, 2, P, C4).transpose(3, 0, 1, 2, 4)
    # 1/area is applied on-device in the g activation scale
    w2T = np.stack([(A(f"s2_{k}")[:, None] * A(f"w2_{k}")).T
                    for k in (1, 3, 5)])
    w2sb = w2T.reshape(3, 4, P, C4).transpose(2, 1, 0, 3)
    # lhsT for d conv: [K=C4, M=C] per k -> [P, 3, C]
    wfT = np.stack([(A(f"sf_{k}")[:, None] * A(f"wf_{k}")).T for k in (1, 3, 5)])
    wfsb = wfT.transpose(1, 0, 2)
    # lhsT for out conv: [K=4C, M=C]; x-part kt 0..3 bf16 [P, 4, C],
    # d-parts kt 4..15 fp8 paired for DoubleRow [P, 3(path), 2(pair), 2, C]
    woT = (A("s_out")[:, None] * A("w_out")).T
    wo_kt = woT.reshape(16, P, C)
    wosb = wo_kt[:4].transpose(1, 0, 2)
    wod8 = wo_kt[4:].reshape(3, 2, 2, P, C).transpose(3, 0, 1, 2, 4)

    b1sb = np.stack([A(f"b1_{k}") for k in (1, 3, 5)]).T
    b2sb = np.stack([A(f"b2_{k}") for k in (1, 3, 5)]).T
    bfsb = np.stack([A(f"bf_{k}").reshape(4, P) for k in (1, 3, 5)]).transpose(2, 0, 1)
    bosb = A("b_out").reshape(4, P).T
    return {
        "w1sb": np.ascontiguousarray(w1sb).astype(ml_dtypes.float8_e4m3),
        "w2sb": np.ascontiguousarray(w2sb).astype(bf16),
        "wfsb": np.ascontiguousarray(wfsb).astype(bf16),
        "wosb": np.ascontiguousarray(wosb).astype(bf16),
        "wod8": np.ascontiguousarray(wod8).astype(ml_dtypes.float8_e4m3),
        "b1sb": np.ascontiguousarray(b1sb),
        "b2sb": np.ascontiguousarray(b2sb),
        "bfsb": np.ascontiguousarray(bfsb),
        "bosb": np.ascontiguousarray(bosb),
        "ident": np.eye(P, dtype=f32).astype(bf16),
    }


def _host_x(x):
    """[512, 3600] fp32 -> partition-major chunked [P, NCHUNK, 4, CHUNK]
    bf16 + fp8, plus the block-transposed pooling copy [P, 4, 16, 225]."""
    xb = x.astype(ml_dtypes.bfloat16)
    # row = kt*128 + p ; col = cb*CHUNK + w
    xc = np.ascontiguousarray(
        xb.reshape(4, P, NCHUNK, CHUNK).transpose(1, 2, 0, 3))
    # (kt, p, hb, r, wb, c) -> [p, kt, (r c), (hb wb)]
    xp = xb.reshape(4, P, 15, 4, 15, 4).transpose(1, 0, 3, 5, 2, 4)
    xp = np.ascontiguousarray(xp.reshape(P, 4, 16, 225))
    return (xc, xc.astype(ml_dtypes.float8_e4m3),
            xp.astype(ml_dtypes.float8_e4m3))


def _run(inputs, **kwargs):
    from concourse.bass_utils import run_bass_kernel_spmd

    common = _host_prep(inputs)
    x = np.asarray(inputs["x"], np.float32).reshape(N_CORES, C, HW)
    in_maps = []
    for n in range(N_CORES):
        xb, x8, xp = _host_x(x[n])
        in_maps.append({**common, "x": xb, "x8": x8, "xp": xp})
    return run_bass_kernel_spmd(_get_nc(), in_maps,
                                core_ids=list(range(N_CORES)), **kwargs)


def kernel(**inputs):
    res = _run(inputs)
    outs = []
    for r in res.results:
        ya = np.asarray(r["y"])   # [P, 4, NB-1, NT]
        y9 = np.asarray(r["y9"])  # [P, 4, NT]
        ya = ya.transpose(1, 0, 2, 3).reshape(C, (NB - 1) * NT)
        y9 = y9.transpose(1, 0, 2).reshape(C, NT)
        outs.append(np.concatenate([ya, y9], axis=1).reshape(C, H, W))
    return np.stack(outs).astype(np.float32)

